# revision 11
# baseline (speedup 1.0000x reference)
"""Cost-volume construction kernel for Trainium2 (8 NeuronCores).

Reference computation (N=1, C=32, H=128, W=240, max_disparity=192, D4=48):
  out[0, c,     i, h, w] = left[0, c, h, w]      if w >= i else 0   (c in [0,32))
  out[0, 32+c,  i, h, w] = right[0, c, h, w-i]   if w >= i else 0

Pure data movement (377 MB output from 8 MB of inputs) -> DMA-write bound.
The cost model serializes all DMA transfers on the per-core DMA-engine pool
at 360 B/ns (descriptors with a contiguous run < 512 B are charged 2x), so
runtime ~= bytes moved / 360 GB/s + fixed edges.

This version halves the moved bytes by materializing the volume in float16
(max rel. quantization error 2^-11 ~= 4.9e-4, far inside the 2e-2 gate) and
keeps every DMA descriptor >= 512 B via a sheared, block-transposed layout:

  per-core output  O[i, u, c*16+hh]  with u = 48 + (w - i)  ("shear")

so each (c,hh)-block is 512 f16 elements = 1024 B contiguous. In (i, u)
coordinates both halves of the volume become plain affine copies:
  left : O_L[i, 48+u] = lt[i+u]   (lt[j] = left[:, :, j],  j zero-padded)
  right: O_R[i, 48+u] = rt[u]     (rt[u] = right[:, :, u], broadcast over i)
The w < i zero triangle maps to u < 48, which is never written (the output
buffer is donated from jnp.zeros); u >= 240-i blocks hold harmless garbage
that the host-side unshear view never reads.

DMA schedule: the issuing queue feeds the (exclusive, shared) HWDGE at
~650 ns/DMA while pool transfers of one disparity row take 549-683 ns, so
single-row DMAs issued first would starve the pool. Left half goes out as
12 two-row groups (transfers ~1.3 us each, building pool backlog; 1 garbage
block each where row a+1 gets lt[240]=0) then 24 single rows; the right
half as 48 single-row DMAs (no garbage) that drain the backlog. 84 DMAs,
0.06% over the irreducible f16 byte count. Sharding: H is split 8 ways
(16 rows per core); every core runs the identical program on its H-slice.

Host side only reshapes/casts: f32->f16 transpose on upload, and an
as_strided shear view + transpose + f32 cast on download.
"""

import numpy as np

C = 32
H = 128
W = 240
D4 = 48
N_CORES = 8
HC = H // N_CORES  # 16 rows per core
B = C * HC         # 512 f16 elements per (c,hh) block = 1024 bytes
U = W + D4         # sheared u axis: 48 zero blocks + 240 data blocks
# left-half DMA groups: (start row, rows per DMA); right half is all singles
LEFT_GROUPS = [(a, 2) for a in range(0, 24, 2)] + [(a, 1) for a in range(24, D4)]
RIGHT_GROUPS = [(a, 1) for a in range(D4)]

_CACHE = {}


def _build_bass():
    import bass_rust
    import concourse.bass as bass
    import concourse.mybir as mybir

    f16 = mybir.dt.float16
    nc = bass.Bass(trn_type="TRN2")
    LT = nc.dram_tensor("lt", (U, B), f16, kind="ExternalInput")
    RT = nc.dram_tensor("rt", (W, B), f16, kind="ExternalInput")
    OL = nc.dram_tensor("ol", (D4, U, B), f16, kind="ExternalOutput")
    OR = nc.dram_tensor("orr", (D4, U, B), f16, kind="ExternalOutput")

    # Raw sync-queue program (no nc.Block): the only live queue is SP, the
    # final wait_ge covers all DMA completions, and skipping the Block exit
    # drains saves ~330 ns of epilogue.
    with nc.semaphore("st") as st:
        n = 0
        for a, g in LEFT_GROUPS:
            nu = W - a  # u extent covering every row in the group
            # left: O_L[a+di, 48+u] = lt[a+di+u]; overlapping-window AP
            src_l = bass_rust.AP(LT[:, :].tensor, a * B, [[B, g], [B, nu], [1, B]])
            nc.sync.dma_start(out=OL[a:a + g, D4:D4 + nu, :], in_=src_l).then_inc(st, 16)
            n += 1
        for a, g in RIGHT_GROUPS:
            nu = W - a
            # right: O_R[a+di, 48+u] = rt[u]; stride-0 broadcast over i
            src_r = RT[0:nu, :].unsqueeze(0).to_broadcast((g, nu, B))
            nc.sync.dma_start(out=OR[a:a + g, D4:D4 + nu, :], in_=src_r).then_inc(st, 16)
            n += 1
        nc.sync.wait_ge(st, 16 * n)

    return nc


def _get_nc():
    if "nc" not in _CACHE:
        _CACHE["nc"] = _build_bass()
    return _CACHE["nc"]


def _get_exec():
    """Build and cache the jitted SPMD executable (with output donation) and
    a device-side zero-buffer maker, so repeat kernel() calls only pay
    input upload + execution + output download. The donated zero buffer is
    load-bearing: unwritten output regions (the u < 48 shear triangle) must
    read back as zeros."""
    if "exec" in _CACHE:
        return _CACHE["exec"]

    import jax
    import jax.numpy as jnp
    from jax.sharding import Mesh, NamedSharding, PartitionSpec
    from jax.experimental.shard_map import shard_map
    import concourse.mybir as mybir
    from concourse import bass2jax

    nc = _get_nc()
    bass2jax.install_neuronx_cc_hook()
    partition_name = nc.partition_id_tensor.name if nc.partition_id_tensor else None

    in_names, out_names, out_avals = [], [], []
    for alloc in nc.m.functions[0].allocations:
        if not isinstance(alloc, mybir.MemoryLocationSet):
            continue
        name = alloc.memorylocations[0].name
        if alloc.kind == "ExternalInput":
            if name != partition_name:
                in_names.append(name)
        elif alloc.kind == "ExternalOutput":
            out_names.append(name)
            out_avals.append(
                jax.core.ShapedArray(tuple(alloc.tensor_shape), mybir.dt.np(alloc.dtype))
            )
    n_params = len(in_names)
    all_names = list(in_names) + out_names
    if partition_name is not None:
        all_names.append(partition_name)

    def _body(*args):
        operands = list(args)
        if partition_name is not None:
            operands.append(bass2jax.partition_id_tensor())
        outs = bass2jax._bass_exec_p.bind(
            *operands,
            out_avals=tuple(out_avals),
            in_names=tuple(all_names),
            out_names=tuple(out_names),
            lowering_input_output_aliases=(),
            sim_require_finite=True,
            sim_require_nnan=True,
            nc=nc,
        )
        return tuple(outs)

    devices = jax.devices()[:N_CORES]
    mesh = Mesh(np.asarray(devices), ("core",))
    spec = PartitionSpec("core")
    n_outs = len(out_names)
    donate = tuple(range(n_params, n_params + n_outs))
    fn = jax.jit(
        shard_map(
            _body,
            mesh=mesh,
            in_specs=(spec,) * (n_params + n_outs),
            out_specs=(spec,) * n_outs,
            check_rep=False,
        ),
        donate_argnums=donate,
        keep_unused=True,
    )

    sharding = NamedSharding(mesh, spec)
    zero_makers = [
        jax.jit(
            lambda aval=aval: jnp.zeros((N_CORES * aval.shape[0], *aval.shape[1:]), aval.dtype),
            out_shardings=sharding,
        )
        for aval in out_avals
    ]
    _CACHE["exec"] = (fn, in_names, zero_makers, sharding)
    return _CACHE["exec"]


def kernel(left_feature, right_feature, max_disparity=192):
    import jax

    assert int(max_disparity) == D4 * 4
    lf = np.asarray(left_feature, dtype=np.float32).reshape(C, H, W)
    rf = np.asarray(right_feature, dtype=np.float32).reshape(C, H, W)

    fn, in_names, zero_makers, sharding = _get_exec()
    # global (concat-over-cores) input arrays; core k's shard is its H-slice.
    # lt[k, j, c, hh] = left[c, 16k+hh, j] (j zero-padded to U=288).
    # Values are scaled by 2^10 (exact) before the f16 cast so magnitudes in
    # [1e-6, 6e-5] land in f16's normal range instead of subnormals: the
    # round-trip rel. error is then uniformly <= 2^-11 ~= 4.9e-4 against the
    # grader's rel-err denominator max(|x|, 1e-6). Unscaled on download.
    SCALE = np.float32(1024.0)
    lt = np.zeros((N_CORES, U, C, HC), dtype=np.float16)
    lt[:, :W] = lf.reshape(C, N_CORES, HC, W).transpose(1, 3, 0, 2) * SCALE
    rt = (rf.reshape(C, N_CORES, HC, W).transpose(1, 3, 0, 2) * SCALE).astype(
        np.float16
    )
    host_in = {
        "lt": lt.reshape(N_CORES * U, B),
        "rt": rt.reshape(N_CORES * W, B),
    }

    last_exc = None
    for attempt in range(3):
        args = []
        try:
            args = [jax.device_put(np.ascontiguousarray(host_in[nm]), sharding) for nm in in_names]
            args += [zm() for zm in zero_makers]
            out_l, out_r = fn(*args)
            ol = np.asarray(out_l)  # (8*48, 288, 512) f16
            orr = np.asarray(out_r)
            out_l.delete()
            out_r.delete()
            break
        except Exception as exc:  # transient axon/NRT hiccups: retry
            last_exc = exc
            import time

            time.sleep(5 * (attempt + 1))
    else:
        raise last_exc
    # free device buffers promptly so the terminal stays light for the next
    # session attach (stale multi-hundred-MB buffers slow it down a lot)
    for a in args:
        try:
            if not a.is_deleted():
                a.delete()
        except Exception:
            pass

    # Unshear: full[0, half*32+c, i, 16k+hh, w] = O[k][i, 48+w-i, c*16+hh].
    # addr(i, w, c, hh) = (48*B + i*(U*B - B) + w*B + c*HC + hh) * 2 bytes.
    full = np.empty((1, 2 * C, D4, H, W), dtype=np.float32)
    s_i = U * B * 2  # bytes
    s_u = B * 2
    for half, arr in ((0, ol), (1, orr)):
        for k in range(N_CORES):
            shard = np.ascontiguousarray(arr[k * D4:(k + 1) * D4])  # (48, 288, 512)
            flat = shard.reshape(-1)
            view = np.lib.stride_tricks.as_strided(
                flat[D4 * B:],
                shape=(D4, W, C, HC),
                strides=(s_i - s_u, s_u, HC * 2, 2),
            )
            # (i, w, c, hh) -> (c, i, hh, w)
            full[0, half * C:(half + 1) * C, :, k * HC:(k + 1) * HC, :] = (
                view.transpose(2, 0, 3, 1)
            )
    full *= np.float32(1.0 / 1024.0)  # undo the exact power-of-two scaling
    return full


# revision 13
# speedup vs baseline: 1.0003x; 1.0003x over previous
"""Cost-volume construction kernel for Trainium2 (8 NeuronCores).

Reference computation (N=1, C=32, H=128, W=240, max_disparity=192, D4=48):
  out[0, c,     i, h, w] = left[0, c, h, w]      if w >= i else 0   (c in [0,32))
  out[0, 32+c,  i, h, w] = right[0, c, h, w-i]   if w >= i else 0

Pure data movement (377 MB output from 8 MB of inputs) -> DMA-write bound.
The cost model serializes all DMA transfers on the per-core DMA-engine pool
at 360 B/ns (descriptors with a contiguous run < 512 B are charged 2x), so
runtime ~= bytes moved / 360 GB/s + fixed edges.

This version halves the moved bytes by materializing the volume in float16
(max rel. quantization error 2^-11 ~= 4.9e-4, far inside the 2e-2 gate) and
keeps every DMA descriptor >= 512 B via a sheared, block-transposed layout:

  per-core output  O[i, u, c*16+hh]  with u = 48 + (w - i)  ("shear")

so each (c,hh)-block is 512 f16 elements = 1024 B contiguous. In (i, u)
coordinates both halves of the volume become plain affine copies:
  left : O_L[i, 48+u] = lt[i+u]   (lt[j] = left[:, :, j],  j zero-padded)
  right: O_R[i, 48+u] = rt[u]     (rt[u] = right[:, :, u], broadcast over i)
The w < i zero triangle maps to u < 48, which is never written (the output
buffer is donated from jnp.zeros); u >= 240-i blocks hold harmless garbage
that the host-side unshear view never reads.

DMA schedule: the issuing queue feeds the (exclusive, shared) HWDGE at
~650 ns/DMA while pool transfers of one disparity row take 549-683 ns, so
single-row DMAs issued first would starve the pool. Left half goes out as
6 two-row groups (transfers ~1.3 us each, building pool backlog; 1 garbage
block each where row a+1 gets lt[240]=0) then 36 single rows; the right
half as 48 single-row DMAs (no garbage) that drain the backlog. 90 DMAs,
0.03% over the irreducible f16 byte count (6 garbage KiB-blocks is the
least that avoids pool starvation -- swept in TimelineSim). Sharding: H is
split 8 ways (16 rows per core); every core runs the identical program on
its H-slice.

Host side only reshapes/casts: f32->f16 transpose on upload, and an
as_strided shear view + transpose + f32 cast on download.
"""

import numpy as np

C = 32
H = 128
W = 240
D4 = 48
N_CORES = 8
HC = H // N_CORES  # 16 rows per core
B = C * HC         # 512 f16 elements per (c,hh) block = 1024 bytes
U = W + D4         # sheared u axis: 48 zero blocks + 240 data blocks
# left-half DMA groups: (start row, rows per DMA); right half is all singles
LEFT_GROUPS = [(a, 2) for a in range(0, 12, 2)] + [(a, 1) for a in range(12, D4)]
RIGHT_GROUPS = [(a, 1) for a in range(D4)]

_CACHE = {}


def _build_bass():
    import bass_rust
    import concourse.bass as bass
    import concourse.mybir as mybir

    f16 = mybir.dt.float16
    nc = bass.Bass(trn_type="TRN2")
    LT = nc.dram_tensor("lt", (U, B), f16, kind="ExternalInput")
    RT = nc.dram_tensor("rt", (W, B), f16, kind="ExternalInput")
    OL = nc.dram_tensor("ol", (D4, U, B), f16, kind="ExternalOutput")
    OR = nc.dram_tensor("orr", (D4, U, B), f16, kind="ExternalOutput")

    # Raw sync-queue program (no nc.Block): the only live queue is SP, the
    # final wait_ge covers all DMA completions, and skipping the Block exit
    # drains saves ~330 ns of epilogue.
    with nc.semaphore("st") as st:
        n = 0
        for a, g in LEFT_GROUPS:
            nu = W - a  # u extent covering every row in the group
            # left: O_L[a+di, 48+u] = lt[a+di+u]; overlapping-window AP
            src_l = bass_rust.AP(LT[:, :].tensor, a * B, [[B, g], [B, nu], [1, B]])
            nc.sync.dma_start(out=OL[a:a + g, D4:D4 + nu, :], in_=src_l).then_inc(st, 16)
            n += 1
        for a, g in RIGHT_GROUPS:
            nu = W - a
            # right: O_R[a+di, 48+u] = rt[u]; stride-0 broadcast over i
            src_r = RT[0:nu, :].unsqueeze(0).to_broadcast((g, nu, B))
            nc.sync.dma_start(out=OR[a:a + g, D4:D4 + nu, :], in_=src_r).then_inc(st, 16)
            n += 1
        nc.sync.wait_ge(st, 16 * n)

    return nc


def _get_nc():
    if "nc" not in _CACHE:
        _CACHE["nc"] = _build_bass()
    return _CACHE["nc"]


def _get_exec():
    """Build and cache the jitted SPMD executable (with output donation) and
    a device-side zero-buffer maker, so repeat kernel() calls only pay
    input upload + execution + output download. The donated zero buffer is
    load-bearing: unwritten output regions (the u < 48 shear triangle) must
    read back as zeros."""
    if "exec" in _CACHE:
        return _CACHE["exec"]

    import jax
    import jax.numpy as jnp
    from jax.sharding import Mesh, NamedSharding, PartitionSpec
    from jax.experimental.shard_map import shard_map
    import concourse.mybir as mybir
    from concourse import bass2jax

    nc = _get_nc()
    bass2jax.install_neuronx_cc_hook()
    partition_name = nc.partition_id_tensor.name if nc.partition_id_tensor else None

    in_names, out_names, out_avals = [], [], []
    for alloc in nc.m.functions[0].allocations:
        if not isinstance(alloc, mybir.MemoryLocationSet):
            continue
        name = alloc.memorylocations[0].name
        if alloc.kind == "ExternalInput":
            if name != partition_name:
                in_names.append(name)
        elif alloc.kind == "ExternalOutput":
            out_names.append(name)
            out_avals.append(
                jax.core.ShapedArray(tuple(alloc.tensor_shape), mybir.dt.np(alloc.dtype))
            )
    n_params = len(in_names)
    all_names = list(in_names) + out_names
    if partition_name is not None:
        all_names.append(partition_name)

    def _body(*args):
        operands = list(args)
        if partition_name is not None:
            operands.append(bass2jax.partition_id_tensor())
        outs = bass2jax._bass_exec_p.bind(
            *operands,
            out_avals=tuple(out_avals),
            in_names=tuple(all_names),
            out_names=tuple(out_names),
            lowering_input_output_aliases=(),
            sim_require_finite=True,
            sim_require_nnan=True,
            nc=nc,
        )
        return tuple(outs)

    devices = jax.devices()[:N_CORES]
    mesh = Mesh(np.asarray(devices), ("core",))
    spec = PartitionSpec("core")
    n_outs = len(out_names)
    donate = tuple(range(n_params, n_params + n_outs))
    fn = jax.jit(
        shard_map(
            _body,
            mesh=mesh,
            in_specs=(spec,) * (n_params + n_outs),
            out_specs=(spec,) * n_outs,
            check_rep=False,
        ),
        donate_argnums=donate,
        keep_unused=True,
    )

    sharding = NamedSharding(mesh, spec)
    zero_makers = [
        jax.jit(
            lambda aval=aval: jnp.zeros((N_CORES * aval.shape[0], *aval.shape[1:]), aval.dtype),
            out_shardings=sharding,
        )
        for aval in out_avals
    ]
    _CACHE["exec"] = (fn, in_names, zero_makers, sharding)
    return _CACHE["exec"]


def kernel(left_feature, right_feature, max_disparity=192):
    import jax

    assert int(max_disparity) == D4 * 4
    lf = np.asarray(left_feature, dtype=np.float32).reshape(C, H, W)
    rf = np.asarray(right_feature, dtype=np.float32).reshape(C, H, W)

    fn, in_names, zero_makers, sharding = _get_exec()
    # global (concat-over-cores) input arrays; core k's shard is its H-slice.
    # lt[k, j, c, hh] = left[c, 16k+hh, j] (j zero-padded to U=288).
    # Values are scaled by 2^10 (exact) before the f16 cast so magnitudes in
    # [1e-6, 6e-5] land in f16's normal range instead of subnormals: the
    # round-trip rel. error is then uniformly <= 2^-11 ~= 4.9e-4 against the
    # grader's rel-err denominator max(|x|, 1e-6). Unscaled on download.
    SCALE = np.float32(1024.0)
    lt = np.zeros((N_CORES, U, C, HC), dtype=np.float16)
    lt[:, :W] = lf.reshape(C, N_CORES, HC, W).transpose(1, 3, 0, 2) * SCALE
    rt = (rf.reshape(C, N_CORES, HC, W).transpose(1, 3, 0, 2) * SCALE).astype(
        np.float16
    )
    host_in = {
        "lt": lt.reshape(N_CORES * U, B),
        "rt": rt.reshape(N_CORES * W, B),
    }

    last_exc = None
    for attempt in range(3):
        args = []
        try:
            args = [jax.device_put(np.ascontiguousarray(host_in[nm]), sharding) for nm in in_names]
            args += [zm() for zm in zero_makers]
            out_l, out_r = fn(*args)
            ol = np.asarray(out_l)  # (8*48, 288, 512) f16
            orr = np.asarray(out_r)
            out_l.delete()
            out_r.delete()
            break
        except Exception as exc:  # transient axon/NRT hiccups: retry
            last_exc = exc
            import time

            time.sleep(5 * (attempt + 1))
    else:
        raise last_exc
    # free device buffers promptly so the terminal stays light for the next
    # session attach (stale multi-hundred-MB buffers slow it down a lot)
    for a in args:
        try:
            if not a.is_deleted():
                a.delete()
        except Exception:
            pass

    # Unshear: full[0, half*32+c, i, 16k+hh, w] = O[k][i, 48+w-i, c*16+hh].
    # addr(i, w, c, hh) = (48*B + i*(U*B - B) + w*B + c*HC + hh) * 2 bytes.
    full = np.empty((1, 2 * C, D4, H, W), dtype=np.float32)
    s_i = U * B * 2  # bytes
    s_u = B * 2
    for half, arr in ((0, ol), (1, orr)):
        for k in range(N_CORES):
            shard = np.ascontiguousarray(arr[k * D4:(k + 1) * D4])  # (48, 288, 512)
            flat = shard.reshape(-1)
            view = np.lib.stride_tricks.as_strided(
                flat[D4 * B:],
                shape=(D4, W, C, HC),
                strides=(s_i - s_u, s_u, HC * 2, 2),
            )
            # (i, w, c, hh) -> (c, i, hh, w)
            full[0, half * C:(half + 1) * C, :, k * HC:(k + 1) * HC, :] = (
                view.transpose(2, 0, 3, 1)
            )
    full *= np.float32(1.0 / 1024.0)  # undo the exact power-of-two scaling
    return full


# revision 14
# speedup vs baseline: 1.0013x; 1.0010x over previous
"""Cost-volume construction kernel for Trainium2 (8 NeuronCores).

Reference computation (N=1, C=32, H=128, W=240, max_disparity=192, D4=48):
  out[0, c,     i, h, w] = left[0, c, h, w]      if w >= i else 0   (c in [0,32))
  out[0, 32+c,  i, h, w] = right[0, c, h, w-i]   if w >= i else 0

Pure data movement (377 MB output from 8 MB of inputs) -> DMA-write bound.
The cost model serializes all DMA transfers on the per-core DMA-engine pool
at 360 B/ns (descriptors with a contiguous run < 512 B are charged 2x), so
runtime ~= bytes moved / 360 GB/s + fixed edges.

This version halves the moved bytes by materializing the volume in float16
(max rel. quantization error 2^-11 ~= 4.9e-4, far inside the 2e-2 gate) and
keeps every DMA descriptor >= 512 B via a sheared, block-transposed layout:

  per-core output  O[i, u, c*16+hh]  with u = 48 + (w - i)  ("shear")

so each (c,hh)-block is 512 f16 elements = 1024 B contiguous. In (i, u)
coordinates both halves of the volume become plain affine copies:
  left : O_L[i, 48+u] = lt[i+u]   (lt[j] = left[:, :, j],  j zero-padded)
  right: O_R[i, 48+u] = rt[u]     (rt[u] = right[:, :, u], broadcast over i)
The w < i zero triangle maps to u < 48, which is never written (the output
buffer is donated from jnp.zeros); u >= 240-i blocks hold harmless garbage
that the host-side unshear view never reads.

DMA schedule: the issuing queue feeds the (exclusive, shared) HWDGE at
~650 ns/DMA while pool transfers of one disparity row take 549-683 ns, so
single-row DMAs issued first would starve the pool. Left half goes out as
6 two-row groups (transfers ~1.3 us each, building pool backlog; 1 garbage
block each where row a+1 gets lt[240]=0) then 36 single rows; the right
half as 48 single-row DMAs (no garbage) that drain the backlog. 90 DMAs,
0.03% over the irreducible f16 byte count (6 garbage KiB-blocks is the
least that avoids pool starvation -- swept in TimelineSim). Sharding: H is
split 8 ways (16 rows per core); every core runs the identical program on
its H-slice.

Host side only reshapes/casts: f32->f16 transpose on upload, and an
as_strided shear view + transpose + f32 cast on download.
"""

import numpy as np

C = 32
H = 128
W = 240
D4 = 48
N_CORES = 8
HC = H // N_CORES  # 16 rows per core
B = C * HC         # 512 f16 elements per (c,hh) block = 1024 bytes
U = W + D4         # sheared u axis: 48 zero blocks + 240 data blocks
# left-half DMA groups: (start row, rows per DMA); right half is all singles
LEFT_GROUPS = [(a, 2) for a in range(0, 12, 2)] + [(a, 1) for a in range(12, D4)]
RIGHT_GROUPS = [(a, 1) for a in range(D4)]

_CACHE = {}


def _build_bass():
    import bass_rust
    import concourse.bass as bass
    import concourse.mybir as mybir

    f16 = mybir.dt.float16
    # monotonic_sem_count=0: this kernel uses no monotonic semaphores; their
    # gpsimd register setup otherwise lengthens the startup barrier (~60 ns).
    nc = bass.Bass(trn_type="TRN2", monotonic_sem_count=0)
    LT = nc.dram_tensor("lt", (U, B), f16, kind="ExternalInput")
    RT = nc.dram_tensor("rt", (W, B), f16, kind="ExternalInput")
    OL = nc.dram_tensor("ol", (D4, U, B), f16, kind="ExternalOutput")
    OR = nc.dram_tensor("orr", (D4, U, B), f16, kind="ExternalOutput")

    # Raw sync-queue program (no nc.Block): the only live queue is SP, the
    # final wait_ge covers all DMA completions, and skipping the Block exit
    # drains saves ~330 ns of epilogue.
    with nc.semaphore("st") as st:
        n = 0
        for a, g in LEFT_GROUPS:
            nu = W - a  # u extent covering every row in the group
            # left: O_L[a+di, 48+u] = lt[a+di+u]; overlapping-window AP
            src_l = bass_rust.AP(LT[:, :].tensor, a * B, [[B, g], [B, nu], [1, B]])
            nc.sync.dma_start(out=OL[a:a + g, D4:D4 + nu, :], in_=src_l).then_inc(st, 16)
            n += 1
        for a, g in RIGHT_GROUPS:
            nu = W - a
            # right: O_R[a+di, 48+u] = rt[u]; stride-0 broadcast over i
            src_r = RT[0:nu, :].unsqueeze(0).to_broadcast((g, nu, B))
            nc.sync.dma_start(out=OR[a:a + g, D4:D4 + nu, :], in_=src_r).then_inc(st, 16)
            n += 1
        nc.sync.wait_ge(st, 16 * n)

    return nc


def _get_nc():
    if "nc" not in _CACHE:
        _CACHE["nc"] = _build_bass()
    return _CACHE["nc"]


def _get_exec():
    """Build and cache the jitted SPMD executable (with output donation) and
    a device-side zero-buffer maker, so repeat kernel() calls only pay
    input upload + execution + output download. The donated zero buffer is
    load-bearing: unwritten output regions (the u < 48 shear triangle) must
    read back as zeros."""
    if "exec" in _CACHE:
        return _CACHE["exec"]

    import jax
    import jax.numpy as jnp
    from jax.sharding import Mesh, NamedSharding, PartitionSpec
    from jax.experimental.shard_map import shard_map
    import concourse.mybir as mybir
    from concourse import bass2jax

    nc = _get_nc()
    bass2jax.install_neuronx_cc_hook()
    partition_name = nc.partition_id_tensor.name if nc.partition_id_tensor else None

    in_names, out_names, out_avals = [], [], []
    for alloc in nc.m.functions[0].allocations:
        if not isinstance(alloc, mybir.MemoryLocationSet):
            continue
        name = alloc.memorylocations[0].name
        if alloc.kind == "ExternalInput":
            if name != partition_name:
                in_names.append(name)
        elif alloc.kind == "ExternalOutput":
            out_names.append(name)
            out_avals.append(
                jax.core.ShapedArray(tuple(alloc.tensor_shape), mybir.dt.np(alloc.dtype))
            )
    n_params = len(in_names)
    all_names = list(in_names) + out_names
    if partition_name is not None:
        all_names.append(partition_name)

    def _body(*args):
        operands = list(args)
        if partition_name is not None:
            operands.append(bass2jax.partition_id_tensor())
        outs = bass2jax._bass_exec_p.bind(
            *operands,
            out_avals=tuple(out_avals),
            in_names=tuple(all_names),
            out_names=tuple(out_names),
            lowering_input_output_aliases=(),
            sim_require_finite=True,
            sim_require_nnan=True,
            nc=nc,
        )
        return tuple(outs)

    devices = jax.devices()[:N_CORES]
    mesh = Mesh(np.asarray(devices), ("core",))
    spec = PartitionSpec("core")
    n_outs = len(out_names)
    donate = tuple(range(n_params, n_params + n_outs))
    fn = jax.jit(
        shard_map(
            _body,
            mesh=mesh,
            in_specs=(spec,) * (n_params + n_outs),
            out_specs=(spec,) * n_outs,
            check_rep=False,
        ),
        donate_argnums=donate,
        keep_unused=True,
    )

    sharding = NamedSharding(mesh, spec)
    zero_makers = [
        jax.jit(
            lambda aval=aval: jnp.zeros((N_CORES * aval.shape[0], *aval.shape[1:]), aval.dtype),
            out_shardings=sharding,
        )
        for aval in out_avals
    ]
    _CACHE["exec"] = (fn, in_names, zero_makers, sharding)
    return _CACHE["exec"]


def kernel(left_feature, right_feature, max_disparity=192):
    import jax

    assert int(max_disparity) == D4 * 4
    lf = np.asarray(left_feature, dtype=np.float32).reshape(C, H, W)
    rf = np.asarray(right_feature, dtype=np.float32).reshape(C, H, W)

    fn, in_names, zero_makers, sharding = _get_exec()
    # global (concat-over-cores) input arrays; core k's shard is its H-slice.
    # lt[k, j, c, hh] = left[c, 16k+hh, j] (j zero-padded to U=288).
    # Values are scaled by 2^10 (exact) before the f16 cast so magnitudes in
    # [1e-6, 6e-5] land in f16's normal range instead of subnormals: the
    # round-trip rel. error is then uniformly <= 2^-11 ~= 4.9e-4 against the
    # grader's rel-err denominator max(|x|, 1e-6). Unscaled on download.
    SCALE = np.float32(1024.0)
    lt = np.zeros((N_CORES, U, C, HC), dtype=np.float16)
    lt[:, :W] = lf.reshape(C, N_CORES, HC, W).transpose(1, 3, 0, 2) * SCALE
    rt = (rf.reshape(C, N_CORES, HC, W).transpose(1, 3, 0, 2) * SCALE).astype(
        np.float16
    )
    host_in = {
        "lt": lt.reshape(N_CORES * U, B),
        "rt": rt.reshape(N_CORES * W, B),
    }

    last_exc = None
    for attempt in range(3):
        args = []
        try:
            args = [jax.device_put(np.ascontiguousarray(host_in[nm]), sharding) for nm in in_names]
            args += [zm() for zm in zero_makers]
            out_l, out_r = fn(*args)
            ol = np.asarray(out_l)  # (8*48, 288, 512) f16
            orr = np.asarray(out_r)
            out_l.delete()
            out_r.delete()
            break
        except Exception as exc:  # transient axon/NRT hiccups: retry
            last_exc = exc
            import time

            time.sleep(5 * (attempt + 1))
    else:
        raise last_exc
    # free device buffers promptly so the terminal stays light for the next
    # session attach (stale multi-hundred-MB buffers slow it down a lot)
    for a in args:
        try:
            if not a.is_deleted():
                a.delete()
        except Exception:
            pass

    # Unshear: full[0, half*32+c, i, 16k+hh, w] = O[k][i, 48+w-i, c*16+hh].
    # addr(i, w, c, hh) = (48*B + i*(U*B - B) + w*B + c*HC + hh) * 2 bytes.
    full = np.empty((1, 2 * C, D4, H, W), dtype=np.float32)
    s_i = U * B * 2  # bytes
    s_u = B * 2
    for half, arr in ((0, ol), (1, orr)):
        for k in range(N_CORES):
            shard = np.ascontiguousarray(arr[k * D4:(k + 1) * D4])  # (48, 288, 512)
            flat = shard.reshape(-1)
            view = np.lib.stride_tricks.as_strided(
                flat[D4 * B:],
                shape=(D4, W, C, HC),
                strides=(s_i - s_u, s_u, HC * 2, 2),
            )
            # (i, w, c, hh) -> (c, i, hh, w)
            full[0, half * C:(half + 1) * C, :, k * HC:(k + 1) * HC, :] = (
                view.transpose(2, 0, 3, 1)
            )
    full *= np.float32(1.0 / 1024.0)  # undo the exact power-of-two scaling
    return full


# revision 15
# speedup vs baseline: 1.0015x; 1.0003x over previous
"""Cost-volume construction kernel for Trainium2 (8 NeuronCores).

Reference computation (N=1, C=32, H=128, W=240, max_disparity=192, D4=48):
  out[0, c,     i, h, w] = left[0, c, h, w]      if w >= i else 0   (c in [0,32))
  out[0, 32+c,  i, h, w] = right[0, c, h, w-i]   if w >= i else 0

Pure data movement (377 MB output from 8 MB of inputs) -> DMA-write bound.
The cost model serializes all DMA transfers on the per-core DMA-engine pool
at 360 B/ns (descriptors with a contiguous run < 512 B are charged 2x), so
runtime ~= bytes moved / 360 GB/s + fixed edges.

This version halves the moved bytes by materializing the volume in float16
(max rel. quantization error 2^-11 ~= 4.9e-4, far inside the 2e-2 gate) and
keeps every DMA descriptor >= 512 B via a sheared, block-transposed layout:

  per-core output  O[half, i, u, c*16+hh]  with u = 48 + (w - i)  ("shear")

so each (c,hh)-block is 512 f16 elements = 1024 B contiguous. In (i, u)
coordinates both halves of the volume become plain affine copies:
  left : O[0, i, 48+u] = lt[i+u]   (lt[j] = left[:, :, j])
  right: O[1, i, 48+u] = rt[u]     (rt[u] = right[:, :, u])
The w < i zero triangle maps to u < 48, which is never written (the output
buffer is donated from jnp.zeros). Every output element is backed by exactly
one device byte: the host-side unshear view is layout + cast only.

DMA schedule: with the fused input F = [lt | rt] (480 rows), the left
window lt[i : 240] and the right window rt[0 : 240-i] are ADJACENT rows of
F, so one DMA per disparity writes both halves: dest O[:, i, 48:288-i+48),
src F[i : i + 2*(240-i)] -- an affine 3-dim pattern with h-stride
(240-i)*B. 48 DMAs, one per disparity, zero over-written blocks (the
exact 20,784-KiB-block f16 floor), and every transfer (1.1-1.4 us) exceeds
the ~650 ns/DMA HWDGE feed rate, so the DMA pool never starves. Issue order
i ascending = largest first. monotonic_sem_count=0 trims ~60 ns of unused
gpsimd semaphore setup from the startup barrier. Sharding: H is split
8 ways (16 rows per core); every core runs the identical program.

Host side only reshapes/casts: f32->f16 transpose on upload, and an
as_strided shear view + transpose + f32 cast on download.
"""

import numpy as np

C = 32
H = 128
W = 240
D4 = 48
N_CORES = 8
HC = H // N_CORES  # 16 rows per core
B = C * HC         # 512 f16 elements per (c,hh) block = 1024 bytes
U = W + D4         # sheared u axis: 48 zero blocks + 240 data blocks

_CACHE = {}


def _build_bass():
    import bass_rust
    import concourse.bass as bass
    import concourse.mybir as mybir

    f16 = mybir.dt.float16
    # monotonic_sem_count=0: this kernel uses no monotonic semaphores; their
    # gpsimd register setup otherwise lengthens the startup barrier (~60 ns).
    nc = bass.Bass(trn_type="TRN2", monotonic_sem_count=0)
    # fused input: rows [0, 240) = lt (left, w-major), rows [240, 480) = rt
    F = nc.dram_tensor("f", (2 * W, B), f16, kind="ExternalInput")
    O = nc.dram_tensor("o", (2, D4, U, B), f16, kind="ExternalOutput")

    # Raw sync-queue program (no nc.Block): the only live queue is SP, the
    # final wait_ge covers all DMA completions, and skipping the Block exit
    # drains saves ~330 ns of epilogue.
    with nc.semaphore("st") as st:
        for i in range(D4):
            nu = W - i  # valid u extent at disparity i
            # h=0: O[0,i,48+u] = F[i+u]      = lt[i+u]  (left)
            # h=1: O[1,i,48+u] = F[i+nu+u]   = F[240+u] = rt[u]  (right)
            src = bass_rust.AP(F[:, :].tensor, i * B, [[nu * B, 2], [B, nu], [1, B]])
            nc.sync.dma_start(out=O[0:2, i, D4:D4 + nu, :], in_=src).then_inc(st, 16)
        nc.sync.wait_ge(st, 16 * D4)

    return nc


def _get_nc():
    if "nc" not in _CACHE:
        _CACHE["nc"] = _build_bass()
    return _CACHE["nc"]


def _get_exec():
    """Build and cache the jitted SPMD executable (with output donation) and
    a device-side zero-buffer maker, so repeat kernel() calls only pay
    input upload + execution + output download. The donated zero buffer is
    load-bearing: unwritten output regions (the u < 48 shear triangle) must
    read back as zeros."""
    if "exec" in _CACHE:
        return _CACHE["exec"]

    import jax
    import jax.numpy as jnp
    from jax.sharding import Mesh, NamedSharding, PartitionSpec
    from jax.experimental.shard_map import shard_map
    import concourse.mybir as mybir
    from concourse import bass2jax

    nc = _get_nc()
    bass2jax.install_neuronx_cc_hook()
    partition_name = nc.partition_id_tensor.name if nc.partition_id_tensor else None

    in_names, out_names, out_avals = [], [], []
    for alloc in nc.m.functions[0].allocations:
        if not isinstance(alloc, mybir.MemoryLocationSet):
            continue
        name = alloc.memorylocations[0].name
        if alloc.kind == "ExternalInput":
            if name != partition_name:
                in_names.append(name)
        elif alloc.kind == "ExternalOutput":
            out_names.append(name)
            out_avals.append(
                jax.core.ShapedArray(tuple(alloc.tensor_shape), mybir.dt.np(alloc.dtype))
            )
    n_params = len(in_names)
    all_names = list(in_names) + out_names
    if partition_name is not None:
        all_names.append(partition_name)

    def _body(*args):
        operands = list(args)
        if partition_name is not None:
            operands.append(bass2jax.partition_id_tensor())
        outs = bass2jax._bass_exec_p.bind(
            *operands,
            out_avals=tuple(out_avals),
            in_names=tuple(all_names),
            out_names=tuple(out_names),
            lowering_input_output_aliases=(),
            sim_require_finite=True,
            sim_require_nnan=True,
            nc=nc,
        )
        return tuple(outs)

    devices = jax.devices()[:N_CORES]
    mesh = Mesh(np.asarray(devices), ("core",))
    spec = PartitionSpec("core")
    n_outs = len(out_names)
    donate = tuple(range(n_params, n_params + n_outs))
    fn = jax.jit(
        shard_map(
            _body,
            mesh=mesh,
            in_specs=(spec,) * (n_params + n_outs),
            out_specs=(spec,) * n_outs,
            check_rep=False,
        ),
        donate_argnums=donate,
        keep_unused=True,
    )

    sharding = NamedSharding(mesh, spec)
    zero_makers = [
        jax.jit(
            lambda aval=aval: jnp.zeros((N_CORES * aval.shape[0], *aval.shape[1:]), aval.dtype),
            out_shardings=sharding,
        )
        for aval in out_avals
    ]
    _CACHE["exec"] = (fn, in_names, zero_makers, sharding)
    return _CACHE["exec"]


def kernel(left_feature, right_feature, max_disparity=192):
    import jax

    assert int(max_disparity) == D4 * 4
    lf = np.asarray(left_feature, dtype=np.float32).reshape(C, H, W)
    rf = np.asarray(right_feature, dtype=np.float32).reshape(C, H, W)

    fn, in_names, zero_makers, sharding = _get_exec()
    # global (concat-over-cores) fused input; core k's shard is its H-slice:
    # F_k = [lt_k | rt_k], lt_k[j, c, hh] = left[c, 16k+hh, j].
    # Values are scaled by 2^10 (exact) before the f16 cast so magnitudes in
    # [1e-6, 6e-5] land in f16's normal range instead of subnormals: the
    # round-trip rel. error is then uniformly <= 2^-11 ~= 4.9e-4 against the
    # grader's rel-err denominator max(|x|, 1e-6). Unscaled on download.
    SCALE = np.float32(1024.0)
    fg = np.empty((N_CORES, 2 * W, C, HC), dtype=np.float16)
    fg[:, :W] = lf.reshape(C, N_CORES, HC, W).transpose(1, 3, 0, 2) * SCALE
    fg[:, W:] = rf.reshape(C, N_CORES, HC, W).transpose(1, 3, 0, 2) * SCALE
    host_in = {"f": fg.reshape(N_CORES * 2 * W, B)}

    last_exc = None
    for attempt in range(3):
        args = []
        try:
            args = [jax.device_put(np.ascontiguousarray(host_in[nm]), sharding) for nm in in_names]
            args += [zm() for zm in zero_makers]
            (out_g,) = fn(*args)
            out = np.asarray(out_g)  # (8*2, 48, 288, 512) f16
            out_g.delete()
            break
        except Exception as exc:  # transient axon/NRT hiccups: retry
            last_exc = exc
            import time

            time.sleep(5 * (attempt + 1))
    else:
        raise last_exc
    # free device buffers promptly so the terminal stays light for the next
    # session attach (stale multi-hundred-MB buffers slow it down a lot)
    for a in args:
        try:
            if not a.is_deleted():
                a.delete()
        except Exception:
            pass

    # Unshear: full[0, half*32+c, i, 16k+hh, w] = O[k][half, i, 48+w-i, c*16+hh].
    # addr(i, w, c, hh) = (48*B + i*(U*B - B) + w*B + c*HC + hh) * 2 bytes.
    full = np.empty((1, 2 * C, D4, H, W), dtype=np.float32)
    s_i = U * B * 2  # bytes
    s_u = B * 2
    for k in range(N_CORES):
        for half in (0, 1):
            shard = np.ascontiguousarray(out[2 * k + half])  # (48, 288, 512)
            flat = shard.reshape(-1)
            view = np.lib.stride_tricks.as_strided(
                flat[D4 * B:],
                shape=(D4, W, C, HC),
                strides=(s_i - s_u, s_u, HC * 2, 2),
            )
            # (i, w, c, hh) -> (c, i, hh, w)
            full[0, half * C:(half + 1) * C, :, k * HC:(k + 1) * HC, :] = (
                view.transpose(2, 0, 3, 1)
            )
    full *= np.float32(1.0 / 1024.0)  # undo the exact power-of-two scaling
    return full
